# revision 1
# baseline (speedup 1.0000x reference)
"""Single-head causal self-attention on 8 trn2 NeuronCores.

Problem: x[4,4096,1024], Wq/Wk/Wv[1024,128]+biases -> causal attention out
[4,4096,128], fp32.

Sharding: core c = (b = c//2, j = c%2). Core (b, j) handles batch b and the
K/V column 128-blocks of parity j (alternating blocks balance the causal
triangle). It computes, for ALL 4096 query rows, the *unnormalized* partial
attention over its own columns plus the exp-sum partials:
    O_un^T[h, s] = sum_{t in cols_j, t<=s} exp(q_s.k_t * scale) * v_t[h]
    ptsum[tau, s] = sum over this core's t-blocks of exp(...)   (t = tau mod 128)
Host combines:  l = ptsum partition-sums;  O = (O_un0 + O_un1)/(l0 + l1) + bv.

Bias algebra (exact):
  - bk is dropped entirely: (q+bq).(k+bk) = (q+bq).k + (q+bq).bk and the
    second term is constant per query row, so it cancels in softmax.
  - bv is applied on the host: sum_t P[t]*(v_t+bv) = sum_t P[t] v_t + bv*l.
  - bq IS applied on the device (it reweights columns).

All SBUF tensors are fp16 (matmul streaming speed on trn2 PE is dtype-
independent at 1 col/cycle, but fp16 halves DMA and doubles DVE rate);
accumulation stays fp32 in PSUM. No per-core max subtraction is needed:
scores are ~N(0,1), so exp never overflows; masked entries get mult-by-0
after exp.

SPMD uniformity: the same Bass program runs on all 8 cores. Parity enters
only through data: for j=1 the host swaps adjacent 128-row blocks of x
(involution), so "even position blocks" on the device are the core's own
columns; the causal masks are per-core inputs. Output comes back in
position space and the host un-swaps.

Device pipeline per core:
  Stage A (per 512-row superstep i2): x^T slice [128, 8, 512] (persistent
    SBUF tile, DMA-sliced on two queues: SP + gpsimd SWDGE) ->
    Q^T = Wq^T x (+bq via ACT bias-copy, fp16), K^T = Wk^T x (even blocks,
    DVE copy), V = x^T.T Wv directly in [t, h] layout (operand-swapped
    matmuls, no PE transpose; DVE copy).
  Stage B (per superblock R of 512 rows, pairs of t-blocks, diagonal pair
    first): S^T[t,s] = K^T.T @ Q^T (PSUM, 2 banks) -> ACT exp -> fp16 pt;
    diagonal pair: DVE mask-mult writes ptsum[:, R] directly; other pairs:
    DVE adds pt into ptsum[:, R]. O^T += V.T @ P^T (PSUM accum over all
    2R+2 t-blocks) -> DVE copy -> DMA. ptsum[:, R] -> DMA (host computes l).
"""

import sys

sys.path.insert(0, "/opt/trn_rl_repo")

import numpy as np

import concourse.bacc as bacc
import concourse.mybir as mybir
import concourse.tile as tile
from concourse import bass_utils

S, E, H, B = 4096, 1024, 128, 4
NSUP, SUP = 8, 512
SCALE = 1.0 / float(np.sqrt(128.0))
F16 = mybir.dt.float16
F32 = mybir.dt.float32
ACT_IDENT = mybir.ActivationFunctionType.Identity
ACT_EXP = mybir.ActivationFunctionType.Exp
MULT = mybir.AluOpType.mult
ADD = mybir.AluOpType.add


def build_nc(loop_n=None):
    nc = bacc.Bacc("TRN2", debug=False, num_devices=8)
    xt_d = nc.dram_tensor("xt", [128, NSUP, 8, SUP], F16, kind="ExternalInput").ap()
    wqb_d = nc.dram_tensor("wqb", [128, 8 * H + 1], F16, kind="ExternalInput").ap()
    wkv_d = nc.dram_tensor("wkv", [128, 2, 8, H], F16, kind="ExternalInput").ap()
    mask_d = nc.dram_tensor("mask", [128, 2, SUP], F16, kind="ExternalInput").ap()
    outT_d = nc.dram_tensor("outT", [H, S], F16, kind="ExternalOutput").ap()
    # NSUP+1 slots: slot NSUP holds the raw exp tile of R7's last pair (its
    # ptsum add is skipped so the slot-7 flush isn't gated on the final AV;
    # the host folds slot 8's column sums into l for R7).
    ps_d = nc.dram_tensor(
        "ptsum", [128, NSUP + 1, 2 * SUP], F16, kind="ExternalOutput"
    ).ap()

    with tile.TileContext(nc) as tc:
        with (
            tc.tile_pool(name="persist", bufs=1) as pp,
            tc.tile_pool(name="pts", bufs=8) as ptp,
            tc.tile_pool(name="ptsums", bufs=NSUP) as psp,
            tc.tile_pool(name="osb", bufs=8) as osp,
            tc.tile_pool(name="psq", bufs=1, space="PSUM") as qpp,
            tc.tile_pool(name="pskv", bufs=1, space="PSUM") as kvp,
            tc.tile_pool(name="psst", bufs=2, space="PSUM") as stp,
            tc.tile_pool(name="pso", bufs=2, space="PSUM") as outp,
        ):
            wqb_s = pp.tile([128, 8 * H + 1], F16, name="wqb")
            wkv_s = pp.tile([128, 2, 8, H], F16, name="wkv")
            mask_s = pp.tile([128, 2, SUP], F16, name="mask")
            qt_all = pp.tile([128, S], F16, name="qt_all")
            kt_all = pp.tile([128, 16, 128], F16, name="kt_all")
            v_all = pp.tile([128, 16, 128], F16, name="v_all")
            xt_s = pp.tile([128, NSUP, 8, SUP], F16, name="xt_s")
            ptsums = {}

            wq_v = wqb_s[:, 0 : 8 * H].rearrange("p (c h) -> p c h", c=8)
            bq_v = wqb_s[:, 8 * H : 8 * H + 1]
            bq32 = pp.tile([128, 1], F32, name="bq32")

            def dma_xt(eng, i2, ca, cb):
                eng.dma_start(
                    xt_s[:, i2, ca:cb, :], xt_d[:, i2, ca:cb, :]
                )

            # PE p-state warm-up: the PE clock ramps to full speed only after
            # ~3us of continuous activity, and the first real matmul can't
            # start until the wqb + xt DMAs land (~3.6us).  Burn the wait on
            # scratch matmuls so the ramp completes before real work arrives.
            warm = pp.tile([128, SUP], F16, name="warm")
            nc.vector.memset(warm, 0.0)
            warm_ps = outp.tile([128, SUP], F32, tag="o", name="warm_ps")
            for _ in range(12):
                nc.tensor.matmul(warm_ps, warm[:, 0:128], warm, start=True, stop=True)

            # The diagonal pair's shrunken exp never writes pt[:, 1, 0:256];
            # the mask multiplies that region by 0, which is only safe if the
            # stale bits are finite (uninit SBUF can decode as fp16 NaN, and
            # NaN*0 = NaN).  Zero the region once on every ring buffer; later
            # reuses hold finite exp values from full-pair writes.
            for _ in range(8):
                pt0 = ptp.tile([128, 2, SUP], F16, tag="pt", name="pt")
                nc.vector.memset(pt0[:, 1, 0:256], 0.0)

            # Startup DMA schedule, tuned so PE streams continuously from
            # ~4us: two issue queues (SP hwdge + gpsimd swdge) in parallel,
            # each DMA costs ~2.7us queue occupancy + transfer, so the two
            # queues alternate supersteps.
            nc.sync.dma_start(wqb_s, wqb_d)
            dma_xt(nc.gpsimd, 0, 0, 4)
            dma_xt(nc.sync, 0, 4, 8)
            dma_xt(nc.gpsimd, 1, 0, 4)
            nc.sync.dma_start(wkv_s, wkv_d)
            dma_xt(nc.sync, 1, 4, 8)
            nc.gpsimd.dma_start(mask_s, mask_d)
            dma_xt(nc.sync, 2, 0, 8)
            dma_xt(nc.gpsimd, 3, 0, 8)
            dma_xt(nc.sync, 4, 0, 8)
            dma_xt(nc.gpsimd, 5, 0, 8)
            dma_xt(nc.sync, 6, 0, 8)
            dma_xt(nc.gpsimd, 7, 0, 8)

            nc.vector.tensor_copy(bq32, bq_v)

            pt_of = {}

            def emit_A(i2):
                xv = xt_s[:, i2]  # [128, 8, 512]
                xv4 = xv.rearrange("p c (t w) -> p c t w", t=4)
                q_ps = qpp.tile([128, SUP], F32, tag="qp", name="q_ps")
                kv_ps = kvp.tile([128, SUP], F32, tag="kvp", name="kv_ps")

                def q_chunks(cs):
                    for c in cs:
                        nc.tensor.matmul(
                            q_ps, wq_v[:, c, :], xv[:, c, :], start=(c == 0), stop=(c == 7)
                        )

                def kv_chunks(cs):
                    for c in cs:
                        nc.tensor.matmul(
                            kv_ps[:, 0:256],
                            wkv_s[:, 0, c, :],
                            xv4[:, c, 0::2, :],
                            start=(c == 0),
                            stop=(c == 7),
                        )
                    # V accumulates into the same PSUM bank as K: start=True
                    # would clear the whole bank's has_written bits (wiping the
                    # other V group / K), so V never uses start — K's c==0
                    # start already cleared the bank, and a start=False matmul
                    # on cleared elements stores rather than accumulates.
                    for c in cs:
                        for t2 in range(2):
                            nc.tensor.matmul(
                                kv_ps[:, 256 + 128 * t2 : 384 + 128 * t2],
                                xv[:, c, 256 * t2 : 256 * t2 + 128],
                                wkv_s[:, 1, c, :],
                                start=False,
                                stop=(c == 7),
                            )
                    if 7 in cs:
                        nc.vector.tensor_copy(
                            kt_all[:, 2 * i2 : 2 * i2 + 2, :],
                            kv_ps[:, 0:256].rearrange("p (t w) -> p t w", t=2),
                        )
                        nc.vector.tensor_copy(
                            v_all[:, 2 * i2 : 2 * i2 + 2, :],
                            kv_ps[:, 256:512].rearrange("p (t w) -> p t w", t=2),
                        )

                def q_finish():
                    nc.vector.tensor_scalar_add(
                        qt_all[:, i2 * SUP : (i2 + 1) * SUP], q_ps, bq32
                    )

                if i2 == 0:
                    # superstep 0: emit in DMA-arrival order (c0-3 half of the
                    # x tile lands first, wkv next, then the c4-7 half)
                    q_chunks(range(4))
                    kv_chunks(range(4))
                    q_chunks(range(4, 8))
                    q_finish()
                    kv_chunks(range(4, 8))
                    return None
                if i2 == NSUP - 1:
                    # final superstep: emit only the Q side; the K/V chunks are
                    # interleaved between R7's first pairs as PE filler for the
                    # exp-paced endgame (returned as deferred closures).
                    q_chunks(range(8))
                    q_finish()

                    def kv_a():
                        kv_chunks(range(4))

                    def kv_b():
                        kv_chunks(range(4, 8))

                    return (kv_a, kv_b)
                q_chunks(range(8))
                q_finish()
                kv_chunks(range(8))
                return None

            o_ps = {}

            def emit_S(task):
                R, p, first_p, last_p = task
                st = stp.tile([128, 2, SUP], F32, tag="st", name="st")
                qv = qt_all[:, R * SUP : (R + 1) * SUP]
                if p == R:
                    # diagonal pair: the 2nd t-block (k=2R+1) has columns
                    # s<256 fully masked for both parities — skip them in the
                    # S matmul and the exp; the mask zeroes whatever is stale
                    # in pt there, and AV skips the region too.
                    nc.tensor.matmul(
                        st[:, 0, :], kt_all[:, 2 * p, :], qv, start=True, stop=True
                    )
                    nc.tensor.matmul(
                        st[:, 1, 256:512],
                        kt_all[:, 2 * p + 1, :],
                        qv[:, 256:512],
                        start=True,
                        stop=True,
                    )
                    pt = ptp.tile([128, 2, SUP], F16, tag="pt", name="pt")
                    nc.scalar.activation(pt[:, 0, :], st[:, 0, :], ACT_EXP, scale=SCALE)
                    nc.scalar.activation(
                        pt[:, 1, 256:512], st[:, 1, 256:512], ACT_EXP, scale=SCALE
                    )
                    if first_p:
                        ptsums[R] = psp.tile([128, 2, SUP], F16, tag="ptsum", name="ptsum")
                        nc.vector.tensor_tensor(ptsums[R], pt, mask_s, MULT)
                        pt_of[(R, p)] = ptsums[R]
                    else:
                        nc.vector.tensor_tensor(pt, pt, mask_s, MULT)
                        pt_of[(R, p)] = pt
                else:
                    for half in range(2):
                        k = 2 * p + half
                        nc.tensor.matmul(
                            st[:, half, :], kt_all[:, k, :], qv, start=True, stop=True
                        )
                    pt = ptp.tile([128, 2, SUP], F16, tag="pt", name="pt")
                    nc.scalar.activation(pt, st, ACT_EXP, scale=SCALE)
                    if first_p:
                        ptsums[R] = psp.tile([128, 2, SUP], F16, tag="ptsum", name="ptsum")
                        nc.vector.tensor_copy(ptsums[R], pt)
                    if last_p and R == NSUP - 1:
                        # raw exp of the final pair -> slot NSUP, straight off
                        # the ACT hwdge queue right behind its own exp
                        nc.scalar.dma_start(ps_d[:, NSUP, :], pt)
                    pt_of[(R, p)] = pt

            def emit_AV(task):
                R, p, first_p, last_p = task
                if first_p:
                    o_ps[R] = outp.tile([128, SUP], F32, tag="o", name="o_ps")
                pt = pt_of.pop((R, p))
                # the ptsum accumulation only needs the exp output; emit it
                # before the AV matmuls so DVE runs it concurrently and the
                # ps DMA isn't gated on the last AV.  R7's final pair skips
                # the add (raw pt was DMA'd as slot NSUP from emit_S), so the
                # slot-7 flush happens at its second-to-last pair.
                final_R = R == NSUP - 1
                if not first_p and not (final_R and last_p):
                    nc.vector.tensor_tensor(ptsums[R], ptsums[R], pt, ADD)
                if (last_p and not final_R) or (final_R and p == 5):
                    nc.gpsimd.dma_start(ps_d[:, R, :], ptsums.pop(R))
                if p == R:
                    # diagonal: 2nd t-block only contributes to s>=256
                    nc.tensor.matmul(
                        o_ps[R], v_all[:, 2 * p, :], pt[:, 0, :],
                        start=first_p, stop=False,
                    )
                    nc.tensor.matmul(
                        o_ps[R][:, 256:512], v_all[:, 2 * p + 1, :],
                        pt[:, 1, 256:512],
                        start=False, stop=last_p,
                    )
                else:
                    for half in range(2):
                        k = 2 * p + half
                        nc.tensor.matmul(
                            o_ps[R],
                            v_all[:, k, :],
                            pt[:, half, :],
                            start=(first_p and half == 0),
                            stop=(last_p and half == 1),
                        )
                if last_p:
                    # superblock R fully accumulated: flush O.  The final
                    # superblock is split into two halves on parallel engine
                    # pairs (ACT-copy -> SP-DMA, DVE-copy -> ACT-DMA) so the
                    # post-compute tail chain is as short as possible.
                    o_sb = osp.tile([128, SUP], F16, tag="o_sb", name="o_sb")
                    nc.vector.tensor_copy(o_sb, o_ps[R])
                    nc.sync.dma_start(outT_d[:, R * SUP : (R + 1) * SUP], o_sb)

            pipe = {"prev": None}

            def push_task(task):
                emit_S(task)
                if pipe["prev"] is not None:
                    emit_AV(pipe["prev"])
                pipe["prev"] = task

            def emit_body():
                pipe["prev"] = None
                pt_of.clear()
                for i2 in range(NSUP):
                    kv_parts = emit_A(i2)
                    if kv_parts is None:
                        # superblock R=i2: pair 0 first — it only needs kt/v
                        # blocks from superstep 0, so it doesn't wait for this
                        # superstep's own DVE copies; the diagonal follows once
                        # those land.
                        order = [0, i2] + list(range(1, i2)) if i2 else [0]
                        for n, p in enumerate(order):
                            push_task((i2, p, n == 0, n == len(order) - 1))
                    else:
                        # final superblock: interleave the deferred K/V(7)
                        # proj chunks between the first pairs so PE has filler
                        # during the exp-paced endgame; diagonal as soon as
                        # K/V(7) lands.
                        kv_a, kv_b = kv_parts
                        order = [0, 1, i2, 2, 3, 4, 5, 6]
                        kv_a()
                        for n, p in enumerate(order):
                            push_task((i2, p, n == 0, n == len(order) - 1))
                            if n == 0:
                                kv_b()
                emit_AV(pipe["prev"])

            if loop_n is None:
                emit_body()
            else:
                with tc.For_i(0, loop_n, 1):
                    emit_body()

    nc.compile()
    return nc


def _perm1():
    idx = np.arange(S)
    return (idx // 128 ^ 1) * 128 + idx % 128


def _mask_for(j):
    ti = np.arange(128)[:, None, None]
    m = np.arange(2)[None, :, None]
    si = np.arange(SUP)[None, None, :]
    orig_s = 128 * ((si // 128) ^ j) + si % 128
    vis = orig_s >= 128 * (2 * m + j) + ti
    return np.where(vis, np.float16(1.0), np.float16(0.0)).astype(np.float16)


def _pack_w(W):
    # [E, H] -> [p, c, h] with E = c*128 + p
    return np.ascontiguousarray(W.reshape(8, 128, H).transpose(1, 0, 2))


_CACHE = {}


def kernel(x, Wq, bq, Wk, bk, Wv, bv):
    if "nc" not in _CACHE:
        _CACHE["nc"] = build_nc()
    nc = _CACHE["nc"]

    x = np.asarray(x, dtype=np.float32)
    Wq = np.asarray(Wq, dtype=np.float32)
    Wk = np.asarray(Wk, dtype=np.float32)
    Wv = np.asarray(Wv, dtype=np.float32)
    bq = np.asarray(bq, dtype=np.float32)
    bk = np.asarray(bk, dtype=np.float32)  # dropped: cancels in softmax
    bv = np.asarray(bv, dtype=np.float32)  # applied on host after combine

    perm = _perm1()
    masks = {j: _mask_for(j) for j in (0, 1)}
    wqb = np.concatenate(
        [_pack_w(Wq).reshape(128, 8 * H), bq.reshape(128, 1)], axis=1
    ).astype(np.float16)
    wkv = np.ascontiguousarray(
        np.stack([_pack_w(Wk), _pack_w(Wv)], axis=1).astype(np.float16)
    )

    blkperm = np.arange(S // 128) ^ 1
    xts = {}
    for b in range(B):
        t = x[b].T.astype(np.float16)  # [E, S]
        for j in (0, 1):
            tj = t if j == 0 else t.reshape(E, S // 128, 128)[:, blkperm, :].reshape(E, S)
            # [E, S] -> [p, i2, c, sl] with E = c*128+p, S = i2*512+sl
            xts[(b, j)] = np.ascontiguousarray(
                tj.reshape(8, 128, NSUP, SUP).transpose(1, 2, 0, 3)
            )

    in_maps = []
    for c in range(8):
        b, j = divmod(c, 2)
        in_maps.append({"xt": xts[(b, j)], "wqb": wqb, "wkv": wkv, "mask": masks[j]})

    res = bass_utils.run_bass_kernel_spmd(nc, in_maps, core_ids=list(range(8)))

    def core_l(r):
        ps = r["ptsum"].astype(np.float32).reshape(128, NSUP + 1, 2, SUP)
        sums = ps.sum(axis=(0, 2))  # [NSUP+1, SUP]
        sums[NSUP - 1] += sums[NSUP]  # fold the raw final-pair slot into R7
        return sums[:NSUP].reshape(S)

    out = np.empty((B, S, H), np.float32)
    for b in range(B):
        r0, r1 = res.results[2 * b], res.results[2 * b + 1]
        oT = r0["outT"].astype(np.float32) + r1["outT"].astype(np.float32)[:, perm]
        l = core_l(r0) + core_l(r1)[perm]
        out[b] = (oT / l[None, :]).T + bv[None, :]
    return out

